# revision 49
# baseline (speedup 1.0000x reference)
"""Trainium2 Bass kernel: GRU decoder with Luong attention (B=32, T=S=512, H=1024, D=80).

Strategy (8 NeuronCores, data-parallel over batch, 4 sequences per core):
  P0: gx = W_ih @ x precomputed for all timesteps -> DRAM (bf16).
  P1: the sequential GRU in transposed layout (H on partitions, batch on the
      free dim); per step 192 bf16 (LDWEIGHTS+MATMUL) pairs + 8 identity-fold
      pairs that add gx_z into PSUM so the z-path sigmoid reads PSUM directly.
      h is written once per step into a persistent bf16 store hB[T] (DVE,
      critical) and an f32 2-slot ring (POOL, recurrence-only).
  P2 is WOVEN into P1: attention + concat/out linears for each finished
      128-step block are emitted as small "filler jobs", one per later step,
      so their matmuls run inside the PE-idle tail of each GRU step instead
      of serially after the loop. All P2 operands are bf16.

All per-core inputs are sliced/transposed on the host; the 8 cores run the
same NEFF via run_bass_kernel_spmd with per-core input maps.
"""

import os
import sys

for _p in ("/opt/trn_rl_repo", "/root/.axon_site/_ro/trn_rl_repo"):
    if os.path.isdir(_p) and _p not in sys.path:
        sys.path.insert(0, _p)

import numpy as np
import ml_dtypes

import concourse.bass as bass
import concourse.mybir as mybir
import concourse.tile as tile
from concourse import bacc
from concourse.bass_utils import run_bass_kernel_spmd
from concourse.masks import make_identity

dt = mybir.dt
AF = mybir.ActivationFunctionType
ALU = mybir.AluOpType

H, D, B, S, T = 1024, 80, 32, 512, 512
BL = 4          # batch per core
NCORES = 8
KC = 8          # H / 128
MC = 24         # 3H / 128
KC2 = 16        # 2H / 128
NBLK = 4        # T / 128 timestep blocks for the woven P2


def _build():
    nc = bacc.Bacc("TRN2", target_bir_lowering=False, debug=False,
                   num_devices=NCORES)
    f32r = dt.float32r

    w_hhT = nc.dram_tensor("w_hhT", [H, 3 * H], dt.bfloat16, kind="ExternalInput")
    w_ihT = nc.dram_tensor("w_ihT", [D, 3 * H], dt.float32, kind="ExternalInput")
    xT = nc.dram_tensor("xT", [D, T * BL], dt.float32, kind="ExternalInput")
    bias_gT = nc.dram_tensor("bias_gT", [128, MC], dt.float32, kind="ExternalInput")
    h0T = nc.dram_tensor("h0T", [128, KC, BL], dt.float32, kind="ExternalInput")
    encT_d = nc.dram_tensor("encT_d", [KC, 128, BL, S], dt.bfloat16,
                            kind="ExternalInput")
    enc_d = nc.dram_tensor("enc_d", [4, 128, BL, H], dt.bfloat16,
                           kind="ExternalInput")
    w_cT = nc.dram_tensor("w_cT", [2 * H, H], dt.bfloat16, kind="ExternalInput")
    b_cT = nc.dram_tensor("b_cT", [128, KC], dt.float32, kind="ExternalInput")
    w_oT = nc.dram_tensor("w_oT", [H, D], dt.bfloat16, kind="ExternalInput")
    b_o_b = nc.dram_tensor("b_o_b", [128, D], dt.float32, kind="ExternalInput")
    maskTb = nc.dram_tensor("maskTb", [128, 4, BL], dt.float32, kind="ExternalInput")
    bhhn = nc.dram_tensor("bhhn", [128, KC, BL], dt.float32, kind="ExternalInput")

    out_l = nc.dram_tensor("out_l", [BL, T, D], dt.float32, kind="ExternalOutput")
    gxT_d = nc.dram_tensor("gxT_d", [MC, 128, T * BL], dt.bfloat16)

    with tile.TileContext(nc) as tc:
        with tc.tile_pool(name="persist", bufs=1) as persist:
            # ---- persistent state / constants ----
            hB = persist.tile([128, KC, T, BL], dt.bfloat16)    # h after step t
            catHr = persist.tile([128, KC, 2, BL], dt.float32)  # f32 h ring
            biasg_sb = persist.tile([128, MC], dt.float32)
            nc.sync.dma_start(out=biasg_sb, in_=bias_gT.ap())
            ident_f = persist.tile([128, 128], dt.float32)
            make_identity(nc, ident_f)
            identB = persist.tile([128, 128], dt.bfloat16)
            nc.vector.tensor_copy(out=identB[:], in_=ident_f[:])
            wc_sb = persist.tile([128, KC2, KC, 128], dt.bfloat16)
            nc.sync.dma_start(
                out=wc_sb,
                in_=w_cT.ap().rearrange("(kc p) (mc m) -> p kc mc m", p=128, m=128))
            encN_sb = persist.tile([128, 4, BL, H], dt.bfloat16)
            nc.sync.dma_start(out=encN_sb, in_=enc_d.ap()
                              .rearrange("sc p b h -> p sc b h"))
            wo_sb = persist.tile([128, KC, D], dt.bfloat16)
            nc.sync.dma_start(out=wo_sb, in_=w_oT.ap()
                              .rearrange("(kc p) d -> p kc d", p=128))
            bo_sb = persist.tile([128, D], dt.float32)
            nc.sync.dma_start(out=bo_sb, in_=b_o_b.ap())
            mask_sb = persist.tile([128, 4, BL], dt.float32)
            nc.sync.dma_start(out=mask_sb, in_=maskTb.ap())
            bc_sb = persist.tile([128, KC], dt.float32)
            nc.sync.dma_start(out=bc_sb, in_=b_cT.ap())
            h0_sb = persist.tile([128, KC, BL], dt.float32)
            nc.sync.dma_start(out=h0_sb, in_=h0T.ap())
            h0B = persist.tile([128, KC, BL], dt.bfloat16)
            nc.vector.tensor_copy(out=h0B[:], in_=h0_sb[:])
            bhhn_sb = persist.tile([128, KC, BL], dt.float32)
            nc.sync.dma_start(out=bhhn_sb, in_=bhhn.ap())

            # ---- P0: gx precompute ----
            with tc.tile_pool(name="p0", bufs=1) as p0, \
                 tc.tile_pool(name="p0o", bufs=4) as p0o, \
                 tc.tile_pool(name="psA", bufs=2, space="PSUM") as psA:
                xT_sb = p0.tile([D, T * BL], dt.float32r)
                nc.sync.dma_start(out=xT_sb, in_=xT.ap().bitcast(f32r))
                wih_sb = p0.tile([D, MC, 128], dt.float32r)
                nc.sync.dma_start(
                    out=wih_sb,
                    in_=w_ihT.ap().bitcast(f32r).rearrange("p (mc m) -> p mc m", m=128))
                for nt in range(4):
                    for mc in range(MC):
                        ps = psA.tile([128, 512], dt.float32, tag="gx")
                        nc.tensor.matmul(ps[:], wih_sb[:, mc, :],
                                         xT_sb[:, nt * 512:(nt + 1) * 512],
                                         start=True, stop=True)
                        gxs = p0o.tile([128, 512], dt.bfloat16, tag="gxo")
                        if (mc + nt) % 2 == 0:
                            nc.scalar.activation(out=gxs[:], in_=ps[:], func=AF.Identity,
                                                 bias=biasg_sb[:, mc:mc + 1], scale=1.0)
                        else:
                            nc.vector.tensor_scalar_add(gxs[:], ps[:],
                                                        biasg_sb[:, mc:mc + 1])
                        nc.sync.dma_start(out=gxT_d.ap()[mc, :, nt * 512:(nt + 1) * 512],
                                          in_=gxs[:])

            # ---- P1 + woven P2 ----
            with tc.tile_pool(name="p1w", bufs=1) as p1w, \
                 tc.tile_pool(name="gxc", bufs=2) as gxcp, \
                 tc.tile_pool(name="p1t", bufs=3) as p1t, \
                 tc.tile_pool(name="zn", bufs=1) as znp, \
                 tc.tile_pool(name="encT", bufs=3) as encTp, \
                 tc.tile_pool(name="p2w", bufs=2) as p2w, \
                 tc.tile_pool(name="p2t", bufs=4) as p2t, \
                 tc.tile_pool(name="p2o", bufs=2) as p2o, \
                 tc.tile_pool(name="psG", bufs=1, space="PSUM") as psG, \
                 tc.tile_pool(name="psB", bufs=1, space="PSUM") as psB:
                w_sb = p1w.tile([128, KC, MC, 128], dt.bfloat16)
                for kc in range(KC):
                    nc.sync.dma_start(
                        out=w_sb[:, kc, :, :],
                        in_=w_hhT.ap()[kc * 128:(kc + 1) * 128, :]
                            .rearrange("p (mc m) -> p mc m", m=128))

                CH = 16
                gx_chunks = []
                for c in range(T // CH):
                    gxc = gxcp.tile([128, MC, CH * BL], dt.bfloat16, tag="gxc")
                    nc.sync.dma_start(out=gxc,
                                      in_=gxT_d.ap().rearrange("mc p c -> p mc c")
                                      [:, :, c * CH * BL:(c + 1) * CH * BL])
                    gx_chunks.append(gxc)

                # ---------- woven-P2 job machinery ----------
                def mk_unit_jobs(b, k):
                    """Filler jobs for attention+linears of (batch b, block k)."""
                    t0 = k * 128
                    st = {}

                    def j_load():
                        st["encT"] = encTp.tile([128, KC, S], dt.bfloat16,
                                                tag="encT", name="encTb")
                        nc.sync.dma_start(
                            out=st["encT"],
                            in_=encT_d.ap().rearrange("kc p b s -> p kc b s")
                            [:, :, b, :])

                    def j_sc(q):
                        def go():
                            if q == 0:
                                st["ps_sc"] = psB.tile([128, S], dt.float32,
                                                       tag="sc", name="ps_sc",
                                                       bufs=2)
                            for kc in range(q * 2, q * 2 + 2):
                                nc.tensor.matmul(
                                    st["ps_sc"][:],
                                    hB[:, kc, t0:t0 + 128, b],
                                    st["encT"][:, kc, :],
                                    start=(kc == 0), stop=(kc == KC - 1))
                        return go

                    def j_sm():
                        negmax = p2t.tile([128, 1], dt.float32, tag="mx")
                        nc.vector.tensor_reduce(negmax[:], st["ps_sc"][:],
                                                axis=mybir.AxisListType.X,
                                                op=ALU.max, negate=True)
                        attn = p2t.tile([128, S], dt.bfloat16, tag="attn")
                        st["attn"] = attn
                        ssum = p2t.tile([128, 1], dt.float32, tag="ssum")
                        nc.scalar.activation(out=attn[:], in_=st["ps_sc"][:],
                                             func=AF.Exp, bias=negmax[:],
                                             scale=1.0, accum_out=ssum[:])
                        rinv = p2t.tile([128, 1], dt.float32, tag="rinv")
                        nc.vector.reciprocal(rinv[:], ssum[:])
                        nc.vector.tensor_scalar_mul(attn[:], attn[:], rinv[:])

                    def j_tr():
                        attnT = p2w.tile([128, 4, 128], dt.bfloat16, tag="attnT")
                        st["attnT"] = attnT
                        for sc in range(4):
                            ps_tr = psB.tile([128, 128], dt.bfloat16, tag="tr")
                            nc.tensor.transpose(
                                ps_tr[:], st["attn"][:, sc * 128:(sc + 1) * 128],
                                identB[:])
                            nc.vector.tensor_copy(out=attnT[:, sc, :], in_=ps_tr[:])

                    def j_ctx(q):
                        def go():
                            if q == 0:
                                st["ctxB"] = p2w.tile([128, KC, 128], dt.bfloat16,
                                                      tag="ctxB", name="ctxB")
                            for hc in range(q * 2, q * 2 + 2):
                                ps_ctx = psB.tile([128, 128], dt.float32, tag="ctx")
                                for sc in range(4):
                                    nc.tensor.matmul(
                                        ps_ctx[:],
                                        encN_sb[:, sc, b, hc * 128:(hc + 1) * 128],
                                        st["attnT"][:, sc, :],
                                        start=(sc == 0), stop=(sc == 3))
                                nc.vector.tensor_copy(out=st["ctxB"][:, hc, :],
                                                      in_=ps_ctx[:])
                        return go

                    def j_cc(mc2, half):
                        def go():
                            if mc2 == 0 and half == 0:
                                st["cTb"] = p2w.tile([128, KC, 128], dt.bfloat16,
                                                     tag="cT", name="cTb")
                            if half == 0:
                                st["ps_c"] = psB.tile([128, 128], dt.float32,
                                                      tag="c", name="ps_c",
                                                      bufs=2)
                                for kc2 in range(KC):
                                    nc.tensor.matmul(
                                        st["ps_c"][:], wc_sb[:, kc2, mc2, :],
                                        hB[:, kc2, t0:t0 + 128, b],
                                        start=(kc2 == 0), stop=False)
                            else:
                                for kc2 in range(KC, KC2):
                                    nc.tensor.matmul(
                                        st["ps_c"][:], wc_sb[:, kc2, mc2, :],
                                        st["ctxB"][:, kc2 - KC, :],
                                        start=False, stop=(kc2 == KC2 - 1))
                                nc.scalar.activation(out=st["cTb"][:, mc2, :],
                                                     in_=st["ps_c"][:], func=AF.Tanh,
                                                     bias=bc_sb[:, mc2:mc2 + 1],
                                                     scale=1.0)
                        return go

                    def j_out():
                        ps_o = psB.tile([128, D], dt.float32, tag="o")
                        for hc in range(KC):
                            nc.tensor.matmul(ps_o[:], st["cTb"][:, hc, :],
                                             wo_sb[:, hc, :],
                                             start=(hc == 0), stop=(hc == KC - 1))
                        o_sb = p2o.tile([128, D], dt.float32, tag="o_s")
                        nc.vector.tensor_add(o_sb[:], ps_o[:], bo_sb[:])
                        nc.vector.tensor_scalar_mul(o_sb[:], o_sb[:],
                                                    mask_sb[:, k, b:b + 1])
                        nc.sync.dma_start(
                            out=out_l.ap()[b, t0:t0 + 128, :], in_=o_sb[:])

                    return (j_load,
                            [j_sc(q) for q in range(4)] + [j_sm, j_tr]
                            + [j_ctx(q) for q in range(4)]
                            + [j_cc(m, hf) for m in range(KC) for hf in (0, 1)]
                            + [j_out])

                def block_jobs(k):
                    # encT loads run ahead of their consumers so the 1MB DMA
                    # is hidden; units are interleaved in pairs so one unit's
                    # softmax/copy stalls are filled by its neighbor's MMs
                    # (matters most for the post-loop drain of block 3).
                    units = [mk_unit_jobs(b, k) for b in range(BL)]
                    out = [units[0][0], units[1][0]]

                    def zip2(ra, rb):
                        z = []
                        for i in range(max(len(ra), len(rb))):
                            if i < len(ra):
                                z.append(ra[i])
                            if i < len(rb):
                                z.append(rb[i])
                        return z

                    out.extend(zip2(units[0][1][:4], units[1][1][:4]))
                    out.append(units[2][0])
                    out.extend(zip2(units[0][1][4:], units[1][1][4:]))
                    out.append(units[3][0])
                    out.extend(zip2(units[2][1], units[3][1]))
                    return out

                jobs = []

                # ---------- the sequential GRU loop ----------
                for t in range(T):
                    c, j = divmod(t, CH)
                    h2 = h0B[:] if t == 0 else hB[:, :, t - 1, :]
                    gxt = gx_chunks[c][:, :, j * BL:(j + 1) * BL]
                    g_all = psG.tile([128, 3, KC, BL], dt.float32, tag="g")
                    for mc in range(KC):
                        for kc in range(KC):
                            nc.tensor.matmul(g_all[:, 0, mc, :], w_sb[:, kc, mc, :],
                                             h2[:, kc, :],
                                             start=(kc == 0), stop=(kc == KC - 1))
                    for mc in range(16, MC):
                        for kc in range(KC):
                            nc.tensor.matmul(g_all[:, 1, mc - 16, :],
                                             w_sb[:, kc, mc, :],
                                             h2[:, kc, :],
                                             start=(kc == 0), stop=(kc == KC - 1))
                    for mc in range(KC, 16):
                        for kc in range(KC):
                            nc.tensor.matmul(g_all[:, 2, mc - KC, :],
                                             w_sb[:, kc, mc, :],
                                             h2[:, kc, :],
                                             start=(kc == 0), stop=False)
                        # fold gx_z into the PSUM accumulation (identity MM):
                        # the z path needs no DVE add and the sigmoid reads
                        # PSUM directly.
                        nc.tensor.matmul(g_all[:, 2, mc - KC, :], identB[:],
                                         gxt[:, mc, :], start=False, stop=True)
                    hold = h0_sb[:] if t == 0 else catHr[:, :, (t - 1) % 2, :]
                    r_s = p1t.tile([128, KC, BL], dt.float32, tag="r_s")
                    nc.vector.tensor_add(r_s[:], g_all[:, 0, :, :], gxt[:, 0:KC, :])
                    nc.scalar.activation(out=r_s[:], in_=r_s[:], func=AF.Sigmoid)
                    # tn1/z_s share one buffer (bufs=1 pool): the WAR forces
                    # tanh before the z sigmoid on the ACT queue.
                    tn1 = znp.tile([128, KC, BL], dt.float32, tag="zn")
                    nc.vector.tensor_add(tn1[:], g_all[:, 1, :, :], bhhn_sb[:])
                    nc.vector.tensor_mul(tn1[:], tn1[:], r_s[:])
                    nc.vector.tensor_add(tn1[:], tn1[:], gxt[:, 16:MC, :])
                    tn = p1t.tile([128, KC, BL], dt.float32, tag="tn")
                    nc.scalar.activation(out=tn[:], in_=tn1[:], func=AF.Tanh)
                    tu = p1t.tile([128, KC, BL], dt.float32, tag="tu")
                    nc.vector.tensor_sub(tu[:], hold, tn[:])
                    z_s = znp.tile([128, KC, BL], dt.float32, tag="zn")
                    nc.scalar.activation(out=z_s[:], in_=g_all[:, 2, :, :],
                                         func=AF.Sigmoid)
                    nc.vector.tensor_mul(z_s[:], z_s[:], tu[:])
                    # h_new: bf16 store (DVE, feeds next step + woven P2) and
                    # f32 ring (POOL, recurrence only)
                    nc.vector.tensor_add(hB[:, :, t, :], z_s[:], tn[:])
                    nc.gpsimd.tensor_add(catHr[:, :, t % 2, :], z_s[:], tn[:])

                    # ---------- woven P2 emission ----------
                    if t % 128 == 127:
                        jobs.extend(block_jobs(t // 128))
                    if t >= 130 and jobs:
                        jobs.pop(0)()

                # drain remaining jobs (block 3 + any stragglers)
                while jobs:
                    jobs.pop(0)()

    nc.compile()
    return nc


def _prep_inputs(inputs, core):
    boff = core * BL
    enc = np.ascontiguousarray(inputs["encoder_outputs"][boff:boff + BL])
    tgt = inputs["target_tensor"][boff:boff + BL]
    tl = inputs["target_length"][boff:boff + BL]
    h0 = inputs["h0"][0, boff:boff + BL]
    W_ih, W_hh = inputs["W_ih"], inputs["W_hh"]
    b_g = (inputs["b_ih"] + inputs["b_hh"]).astype(np.float32)
    b_g[2 * H:] = inputs["b_ih"][2 * H:]   # b_hh_n goes inside the r-multiply
    bhhn_np = np.broadcast_to(
        inputs["b_hh"][2 * H:].reshape(KC, 128).T[:, :, None], (128, KC, BL)).copy()

    xs = np.concatenate([np.zeros((1, BL, D), np.float32),
                         tgt.transpose(1, 0, 2)[:-1]], 0)
    xT = np.ascontiguousarray(xs.reshape(T * BL, D).T)

    return {
        "w_hhT": np.ascontiguousarray(W_hh.T).astype(ml_dtypes.bfloat16),
        "w_ihT": np.ascontiguousarray(W_ih.T),
        "xT": xT,
        "bias_gT": np.ascontiguousarray(b_g.reshape(MC, 128).T),
        "h0T": np.ascontiguousarray(h0.T.reshape(KC, 128, BL).transpose(1, 0, 2)),
        "encT_d": np.ascontiguousarray(
            enc.transpose(2, 1, 0).reshape(KC, 128, S, BL).transpose(0, 1, 3, 2))
            .astype(ml_dtypes.bfloat16),
        "enc_d": np.ascontiguousarray(
            enc.transpose(1, 0, 2).reshape(4, 128, BL, H)).astype(ml_dtypes.bfloat16),
        "w_cT": np.ascontiguousarray(inputs["W_c"].T).astype(ml_dtypes.bfloat16),
        "b_cT": np.ascontiguousarray(inputs["b_c"].reshape(KC, 128).T),
        "w_oT": np.ascontiguousarray(inputs["W_o"].T).astype(ml_dtypes.bfloat16),
        "b_o_b": np.broadcast_to(inputs["b_o"], (128, D)).copy(),
        "maskTb": np.ascontiguousarray(
            (np.arange(T)[:, None] < tl[None, :]).astype(np.float32)
            .reshape(4, 128, BL).transpose(1, 0, 2)),
        "bhhn": bhhn_np,
    }


_NC_CACHE = []
LAST_EXEC_NS = None


def _install_trace_shim():
    """antenv.axon_hooks shim so trace=True works under axon in this container."""
    import types, ctypes, contextlib
    if "antenv.axon_hooks" in sys.modules:
        return
    so_path = "/opt/axon/libaxon_pjrt.so"
    hook = None
    if os.path.exists(so_path):
        lib = ctypes.CDLL(so_path)
        if hasattr(lib, "axon_start_nrt_profile"):
            lib.axon_start_nrt_profile.argtypes = [ctypes.POINTER(ctypes.c_int64),
                                                   ctypes.c_size_t]
            lib.axon_start_nrt_profile.restype = ctypes.c_int64
            lib.axon_stop_nrt_profile.argtypes = [ctypes.c_char_p]
            lib.axon_stop_nrt_profile.restype = ctypes.c_int64

            @contextlib.contextmanager
            def _hook(output_dir, device_ids):
                import jax
                jax.devices()
                if device_ids:
                    ids = (ctypes.c_int64 * len(device_ids))(*device_ids)
                    rc = lib.axon_start_nrt_profile(ids, len(device_ids))
                else:
                    rc = lib.axon_start_nrt_profile(None, 0)
                if rc != 0:
                    raise RuntimeError(f"axon_start_nrt_profile rc={rc}")
                try:
                    yield
                finally:
                    n = lib.axon_stop_nrt_profile(str(output_dir).encode())
                    print(f"profile: {n} file(s) written to {output_dir}",
                          file=sys.stderr)
            hook = _hook
    mod = types.ModuleType("antenv.axon_hooks")
    mod.get_axon_ntff_profile_hook = lambda: hook
    mod.set_axon_ntff_profile_hook = lambda h: None
    sys.modules["antenv.axon_hooks"] = mod
    import concourse.bass_utils as bu
    bu.upload_artifacts = lambda tmpdir: f"local://{tmpdir}"


def kernel(**inputs):
    global LAST_EXEC_NS
    inputs = {k: np.asarray(v) for k, v in inputs.items()}
    if not _NC_CACHE:
        _NC_CACHE.append(_build())
    nc = _NC_CACHE[0]
    in_maps = [_prep_inputs(inputs, core) for core in range(NCORES)]
    kwargs = {}
    if os.environ.get("DEC_TRACE") == "1":
        _install_trace_shim()
        import tempfile
        kwargs = dict(trace=True, tmpdir=tempfile.mkdtemp(prefix="dec_trace_"))
    res = run_bass_kernel_spmd(nc, in_maps, core_ids=list(range(NCORES)), **kwargs)
    LAST_EXEC_NS = res.exec_time_ns
    out = np.concatenate([res.results[c]["out_l"] for c in range(NCORES)], axis=0)
    return out.astype(np.float32)
